# revision 25
# baseline (speedup 1.0000x reference)
"""Trainium2 Bass kernel for additive attention (nn_AdditiveAttention).

Reference computation (per batch b):
    q_proj = query @ W1_w.T + W1_b                      # [D]
    v_proj = values @ W2_w.T + W2_b                     # [T, D]
    scores = tanh(q_proj + v_proj) @ v                  # [T]
    weights = softmax(scores)                           # [T]
    out    = weights @ values                           # [E]

Sharding: data-parallel over batch B=32 across 8 NeuronCores (4 batches/core).

V6 design. `values` is consumed ONLY in transposed [e, t] layout (a single
host-pretransposed contiguous 32MB bf16 stream per core; ~392 GB/s measured
vs ~166 GB/s for the DMA-XBAR transpose path). Per group of 2048 timesteps:

  - DMA vt [128e, 4, 2048t] bf16 (one 2MB load)
  - v_proj: 4 psum tiles [128d(chunk dc), 512t], dc-outer / e-chunk-inner so
    each W2 chunk is loaded once per pass and reused for 4 super matmuls
  - ACT tanh w/ per-partition bias -> th bf16
  - scores: 8 MMs (lhsT = v chunk [128,1]) -> rows {0,32} of two [64, 512]
    psum tiles
  - softmax WITHOUT max subtraction (|scores| <~ 25 so exp fits f32 easily;
    partial num/den per group are combined exactly on the host):
      ACT exp on the score rows (PSUM->SBUF, accum_out = per-row partial den)
  - weights row [1, 2048] -> all partitions via 4 GPSIMD partition_broadcast
  - numerator on the DVE: 4x tensor_tensor_reduce
      num_col[p, c] = sum_t vt[p, c, t] * w_bcast[p, t]   (e = c*128+p)
    This removes the 256 numerator matmuls, the score-transposes and the
    GPSIMD all-reduces from the critical PE path, and the entire natural-
    layout values stream from DMA.
  - per (batch, group) output tile [128, 6]: num cols + den partials -> host

Engine budget per core: PE ~136us (v_proj 109 + scores 27), DVE ~143us
(ttr), ACT ~97us (tanh+exp), DMA ~90us (32MB), Pool ~30us (broadcasts).

All matmul operands are bf16 (~4e-3 rel err; harness gate is 2e-2).
"""

import os
import sys
import time

import numpy as np

for _p in ("/opt/trn_rl_repo",):
    if _p not in sys.path and os.path.isdir(_p):
        sys.path.insert(0, _p)

# Problem shapes (hardcoded per contract)
B, T, E, D = 32, 8192, 512, 256
N_CORES = 8
B_LOC = B // N_CORES          # 4 batches per core
P = 128
TSUP = 512                    # timesteps per super tile
JSUB = TSUP // P              # 4 basic 128-t subtiles per super
SUP_PER_GROUP = 4             # supers per softmax group
T_GROUP = TSUP * SUP_PER_GROUP  # 2048
EC = E // P                   # 4 e-chunks
DC = D // P                   # 2 d-chunks
NB = SUP_PER_GROUP * JSUB     # 16 basic 128-t tiles per group
OUTW = EC + 4                 # num cols [4] + 4 den partials (row 0)

LAST_RESULT = None            # BassKernelResults of the most recent run


def build_bass(t_loc=T, b_loc=B_LOC, vpool_bufs=4, repeat=1, loop_n=1):
    """Build the Bass module (same SPMD program for every core)."""
    import concourse.bacc as bacc
    import concourse.tile as tile
    from concourse import mybir

    f32 = mybir.dt.float32
    dtm = mybir.dt.bfloat16

    n_groups = t_loc // T_GROUP
    assert t_loc % T_GROUP == 0

    nc = bacc.Bacc("TRN2", target_bir_lowering=False, debug=False,
                   num_devices=N_CORES)
    valsT = nc.dram_tensor("valuesT", [b_loc, E, t_loc], dtm,
                           kind="ExternalInput").ap()
    w2ed_d = nc.dram_tensor("w2ed", [E, D], dtm, kind="ExternalInput").ap()
    cb_d = nc.dram_tensor("cb", [D, b_loc], f32, kind="ExternalInput").ap()
    v_d = nc.dram_tensor("vcol", [D, 1], dtm, kind="ExternalInput").ap()
    outp = nc.dram_tensor("out_parts", [b_loc, n_groups, P, OUTW], f32,
                          kind="ExternalOutput").ap()

    with tile.TileContext(nc) as tc:
        _emit(tc, valsT, w2ed_d, cb_d, v_d, outp, b_loc,
              n_groups, vpool_bufs, repeat, loop_n, dtm)
    nc.compile()
    return nc


def _emit(tc, valsT, w2ed_d, cb_d, v_d, outp, b_loc,
          n_groups, vpool_bufs, repeat, loop_n, dtm):
    from contextlib import ExitStack

    from concourse import mybir

    f32 = mybir.dt.float32
    Tanh = mybir.ActivationFunctionType.Tanh
    Exp = mybir.ActivationFunctionType.Exp
    Mult = mybir.AluOpType.mult
    Add = mybir.AluOpType.add

    nc = tc.nc

    with ExitStack() as ctx:
        consts = ctx.enter_context(tc.tile_pool(name="consts", bufs=1))
        vtpool = ctx.enter_context(
            tc.tile_pool(name="vtpool", bufs=vpool_bufs))
        thpool = ctx.enter_context(tc.tile_pool(name="thpool", bufs=18))
        wpool = ctx.enter_context(tc.tile_pool(name="wpool", bufs=2))
        bpool = ctx.enter_context(tc.tile_pool(name="bpool", bufs=2))
        scrpool = ctx.enter_context(tc.tile_pool(name="scrpool", bufs=2))
        opool = ctx.enter_context(tc.tile_pool(name="opool", bufs=4))
        ps_vp = ctx.enter_context(
            tc.tile_pool(name="ps_vp", bufs=4, space="PSUM"))
        ps_sm = ctx.enter_context(
            tc.tile_pool(name="ps_sm", bufs=4, space="PSUM"))

        # GPSIMD ucode library providing partition_broadcast
        from concourse import library_config
        nc.gpsimd.load_library(library_config.mlp)

        # --- constants ---
        w2_sb = consts.tile([P, EC, D], dtm)
        nc.sync.dma_start(w2_sb, w2ed_d.rearrange("(c p) d -> p c d", p=P))
        cb_sb = consts.tile([P, DC, b_loc], f32)
        nc.sync.dma_start(cb_sb, cb_d.rearrange("(c p) b -> p c b", p=P))
        v_sb = consts.tile([P, DC, 1], dtm)
        nc.sync.dma_start(v_sb, v_d.rearrange("(c p) x -> p c x", p=P))

        def emit_weights(p):
            """Prev group's exp + den partials (ACT) -> broadcast rows.

            partition_broadcast reads physical partition 0 of its source
            (AP partition offsets are ignored), so every score row lives at
            partition 0 of its own psum bank / SBUF tile."""
            psss, b, g, rep = p
            denp = opool.tile([1, SUP_PER_GROUP], f32, tag="dp",
                              name=f"dp_{rep}_{b}_{g}")
            wb = bpool.tile([P, SUP_PER_GROUP, TSUP], dtm, tag="wb",
                            name=f"wb_{rep}_{b}_{g}")

            def half(srange):
                for s in srange:
                    wrow = wpool.tile([1, TSUP], dtm, tag=f"wr{s}",
                                      name=f"wr{s}_{rep}_{b}_{g}")
                    nc.scalar.activation(wrow, psss[s], Exp,
                                         accum_out=denp[:, s:s + 1])
                    nc.gpsimd.partition_broadcast(
                        wb[:, s, :], wrow, channels=P)
            return wb, denp, half

        def emit_numerator(p, wb, denp):
            """Prev group's numerator: 4 DVE fused mult+reduce over t."""
            vt, b, g, rep = p["vt"], p["b"], p["g"], p["rep"]
            osb = opool.tile([P, OUTW], f32, tag="osb",
                             name=f"osb_{rep}_{b}_{g}")
            scr = scrpool.tile([P, T_GROUP], dtm, tag="scr",
                               name=f"scr_{rep}_{b}_{g}")
            for c in range(EC):
                nc.vector.scalar_tensor_tensor(
                    out=scr, in0=vt[:, c, :], scalar=1.0,
                    in1=wb.rearrange("p s t -> p (s t)"),
                    op0=Mult, op1=Mult,
                    accum_out=osb[:, c:c + 1])
            nc.vector.tensor_copy(osb[0:1, EC:], denp)
            nc.sync.dma_start(outp[b, g], osb)

        def emit_scores(p):
            """Prev group's score MMs: deferred a full group so the PE
            never waits on that group's tanh latency."""
            for s in range(SUP_PER_GROUP):
                nc.tensor.matmul(p["psss"][s], lhsT=v_sb[:, 0, :],
                                 rhs=p["ths"][s][0], start=True, stop=False)
                nc.tensor.matmul(p["psss"][s], lhsT=v_sb[:, 1, :],
                                 rhs=p["ths"][s][1], start=False, stop=True)

        def body(rep):
          prev = None
          for b in range(b_loc):
            for g in range(n_groups):
                t0g = g * T_GROUP
                vt = vtpool.tile([P, EC, T_GROUP], dtm, tag="vt",
                                 name=f"vt_{rep}_{b}_{g}")
                for c in range(EC):
                    nc.sync.dma_start(
                        vt[:, c, :],
                        valsT[b, c * P:(c + 1) * P,
                              t0g:t0g + T_GROUP])

                # scores psum: one single-row bank per super (row 0, so
                # the downstream partition_broadcast sees partition 0)
                psss = [ps_sm.tile([1, TSUP], f32, tag="scrow",
                                   name=f"pss{s}_{rep}_{b}_{g}")
                        for s in range(SUP_PER_GROUP)]

                # ---- v_proj, dc-outer so each W2 chunk is loaded into the
                # PE once per pass and reused for 4 super-matmuls ----------
                ths = [[None] * DC for _ in range(SUP_PER_GROUP)]
                psvs = [None] * SUP_PER_GROUP
                # Prev group's deferred score MMs first: they fill the
                # PE while this group's first psv bank waits on the prev
                # dc1 tanh (their own tanh inputs are a full group old).
                if prev is not None:
                    emit_scores(prev)
                for dc in range(DC):
                    for c in range(EC):
                        for s in range(SUP_PER_GROUP):
                            if c == 0:
                                psvs[s] = ps_vp.tile(
                                    [P, TSUP], f32, tag="psv",
                                    name=f"psv_{rep}_{b}_{g}_{s}_{dc}")
                            nc.tensor.matmul(
                                psvs[s],
                                lhsT=w2_sb[:, c, dc * P:(dc + 1) * P],
                                rhs=vt[:, c, s * TSUP:(s + 1) * TSUP],
                                start=(c == 0), stop=(c == EC - 1))
                            if c == EC - 1:
                                th = thpool.tile(
                                    [P, TSUP], dtm, tag="th",
                                    name=f"th_{rep}_{b}_{g}_{s}_{dc}")
                                nc.scalar.activation(
                                    th, psvs[s], Tanh,
                                    bias=cb_sb[:, dc, b:b + 1])
                                ths[s][dc] = th
                    # Prev group's softmax tail: half the exps sit between
                    # the two weight passes (so the numerator chain starts),
                    # half after dc1's tanhs (so those tanhs are not queued
                    # behind 4 exps - the next group's first psv bank waits
                    # on them).
                    if prev is not None:
                        if dc == 0:
                            wb_p, denp_p, half_p = emit_weights(prev["soft"])
                            half_p(range(0, 2))
                        else:
                            half_p(range(2, SUP_PER_GROUP))
                            emit_numerator(prev, wb_p, denp_p)
                            prev = None

                prev = {"vt": vt, "b": b, "g": g, "rep": rep, "ths": ths,
                        "psss": psss, "soft": (psss, b, g, rep)}
          emit_scores(prev)
          wb_p, denp_p, half_p = emit_weights(prev["soft"])
          half_p(range(SUP_PER_GROUP))
          emit_numerator(prev, wb_p, denp_p)

        if loop_n > 1:
            with tc.For_i(0, loop_n, 1):
                for rep in range(repeat):
                    body(rep)
        else:
            for rep in range(repeat):
                body(rep)


def host_prepare(values, query, v, W1_w, W1_b, W2_w, W2_b, b_loc=B_LOC,
                 n_cores=N_CORES):
    """Precompute tiny host-side tensors and build per-core input maps."""
    import ml_dtypes

    npm = ml_dtypes.bfloat16

    c = (query.astype(np.float32) @ W1_w.T.astype(np.float32)
         + W1_b + W2_b).astype(np.float32)          # [B, D]
    values_m = np.asarray(values).astype(npm)
    values_t = np.ascontiguousarray(values_m.transpose(0, 2, 1))  # [B, E, T]
    w2ed = np.ascontiguousarray(np.asarray(W2_w).T.astype(npm))  # [E, D]
    vcol = np.ascontiguousarray(np.asarray(v).reshape(D, 1).astype(npm))
    in_maps = []
    for k in range(n_cores):
        bsl = slice(k * b_loc, (k + 1) * b_loc)
        in_maps.append({
            "valuesT": np.ascontiguousarray(values_t[bsl]),
            "w2ed": w2ed,
            "cb": np.ascontiguousarray(c[bsl].T),    # [D, b_loc]
            "vcol": vcol,
        })
    return in_maps


def host_combine(results, b_loc=B_LOC, n_cores=N_CORES):
    """Combine per-(batch, group) partial numerators/denominators.

    out_parts[b, g, p, c] for c<EC is sum_t w[t] * values[t, c*128+p];
    row 0 cols EC..EC+3 hold the per-super partial denominators.
    No max-subtraction: partials are exact exp sums (safe in f32 range).
    """
    out = np.zeros((n_cores * b_loc, E), np.float32)
    for k in range(n_cores):
        parts = np.asarray(results[k]["out_parts"]).astype(np.float64)
        num = parts[..., :EC]                       # [b, g, 128, EC]
        den = parts[..., 0, EC:].sum(-1)            # [b, g]
        numf = num.transpose(0, 1, 3, 2).reshape(b_loc, -1, E)  # e=c*128+p
        o = numf.sum(1) / den.sum(1)[:, None]
        out[k * b_loc:(k + 1) * b_loc] = o.astype(np.float32)
    return out


_NC_CACHE = None


def kernel(values, query, v, W1_w, W1_b, W2_w, W2_b):
    global _NC_CACHE, LAST_RESULT
    from concourse.bass_utils import run_bass_kernel_spmd

    in_maps = host_prepare(values, query, v, W1_w, W1_b, W2_w, W2_b)
    if _NC_CACHE is None:
        _NC_CACHE = build_bass()
    trace = bool(int(os.environ.get("KERNEL_TRACE", "0")))
    LAST_RESULT = run_bass_kernel_spmd(
        _NC_CACHE, in_maps, list(range(N_CORES)), trace=trace)
    return host_combine(LAST_RESULT.results)


if __name__ == "__main__":
    rng = np.random.default_rng(0)
    inputs = {
        "values": rng.standard_normal((B, T, E), dtype=np.float32),
        "query": rng.standard_normal((B, D), dtype=np.float32),
        "v": rng.random(D, dtype=np.float32),
        "W1_w": rng.standard_normal((D, D), dtype=np.float32) * 0.06,
        "W1_b": rng.standard_normal(D, dtype=np.float32) * 0.06,
        "W2_w": rng.standard_normal((D, E), dtype=np.float32) * 0.04,
        "W2_b": rng.standard_normal(D, dtype=np.float32) * 0.04,
    }
    t0 = time.time()
    out = kernel(**inputs)
    print("kernel done in", time.time() - t0, "s", out.shape, out.dtype)


# revision 26
# speedup vs baseline: 1.1445x; 1.1445x over previous
"""Trainium2 Bass kernel for additive attention (nn_AdditiveAttention).

Reference computation (per batch b):
    q_proj = query @ W1_w.T + W1_b                      # [D]
    v_proj = values @ W2_w.T + W2_b                     # [T, D]
    scores = tanh(q_proj + v_proj) @ v                  # [T]
    weights = softmax(scores)                           # [T]
    out    = weights @ values                           # [E]

Sharding: data-parallel over batch B=32 across 8 NeuronCores (4 batches/core).

V6 design. `values` is consumed ONLY in transposed [e, t] layout (a single
host-pretransposed contiguous 32MB bf16 stream per core; ~392 GB/s measured
vs ~166 GB/s for the DMA-XBAR transpose path). Per group of 2048 timesteps:

  - DMA vt [128e, 4, 2048t] bf16 (one 2MB load)
  - v_proj: 4 psum tiles [128d(chunk dc), 512t], dc-outer / e-chunk-inner so
    each W2 chunk is loaded once per pass and reused for 4 super matmuls
  - ACT tanh w/ per-partition bias -> th bf16
  - scores: 8 MMs (lhsT = v chunk [128,1]) -> rows {0,32} of two [64, 512]
    psum tiles
  - softmax WITHOUT max subtraction (|scores| <~ 25 so exp fits f32 easily;
    partial num/den per group are combined exactly on the host):
      ACT exp on the score rows (PSUM->SBUF, accum_out = per-row partial den)
  - weights row [1, 2048] -> all partitions via 4 GPSIMD partition_broadcast
  - numerator on the DVE: 4x tensor_tensor_reduce
      num_col[p, c] = sum_t vt[p, c, t] * w_bcast[p, t]   (e = c*128+p)
    This removes the 256 numerator matmuls, the score-transposes and the
    GPSIMD all-reduces from the critical PE path, and the entire natural-
    layout values stream from DMA.
  - per (batch, group) output tile [128, 6]: num cols + den partials -> host

Engine budget per core: PE ~136us (v_proj 109 + scores 27), DVE ~143us
(ttr), ACT ~97us (tanh+exp), DMA ~90us (32MB), Pool ~30us (broadcasts).

All matmul operands are bf16 (~4e-3 rel err; harness gate is 2e-2).
"""

import os
import sys
import time

import numpy as np

for _p in ("/opt/trn_rl_repo",):
    if _p not in sys.path and os.path.isdir(_p):
        sys.path.insert(0, _p)

# Problem shapes (hardcoded per contract)
B, T, E, D = 32, 8192, 512, 256
N_CORES = 8
B_LOC = B // N_CORES          # 4 batches per core
P = 128
TSUP = 512                    # timesteps per super tile
JSUB = TSUP // P              # 4 basic 128-t subtiles per super
SUP_PER_GROUP = 4             # supers per softmax group
T_GROUP = TSUP * SUP_PER_GROUP  # 2048
EC = E // P                   # 4 e-chunks
DC = D // P                   # 2 d-chunks
NB = SUP_PER_GROUP * JSUB     # 16 basic 128-t tiles per group
OUTW = EC + 4                 # num cols [4] + 4 den partials (row 0)

LAST_RESULT = None            # BassKernelResults of the most recent run


def build_bass(t_loc=T, b_loc=B_LOC, vpool_bufs=4, repeat=1, loop_n=1):
    """Build the Bass module (same SPMD program for every core)."""
    import concourse.bacc as bacc
    import concourse.tile as tile
    from concourse import mybir

    f32 = mybir.dt.float32
    dtm = mybir.dt.bfloat16

    n_groups = t_loc // T_GROUP
    assert t_loc % T_GROUP == 0

    nc = bacc.Bacc("TRN2", target_bir_lowering=False, debug=False,
                   num_devices=N_CORES)
    valsT = nc.dram_tensor("valuesT", [b_loc, E, t_loc], dtm,
                           kind="ExternalInput").ap()
    w2ed_d = nc.dram_tensor("w2ed", [E, D], dtm, kind="ExternalInput").ap()
    cb_d = nc.dram_tensor("cb", [D, b_loc], f32, kind="ExternalInput").ap()
    v_d = nc.dram_tensor("vcol", [D, 1], dtm, kind="ExternalInput").ap()
    outp = nc.dram_tensor("out_parts", [b_loc, n_groups, P, OUTW], f32,
                          kind="ExternalOutput").ap()

    with tile.TileContext(nc) as tc:
        _emit(tc, valsT, w2ed_d, cb_d, v_d, outp, b_loc,
              n_groups, vpool_bufs, repeat, loop_n, dtm)
    nc.compile()
    return nc


def _emit(tc, valsT, w2ed_d, cb_d, v_d, outp, b_loc,
          n_groups, vpool_bufs, repeat, loop_n, dtm):
    from contextlib import ExitStack

    from concourse import mybir

    f32 = mybir.dt.float32
    Tanh = mybir.ActivationFunctionType.Tanh
    Exp = mybir.ActivationFunctionType.Exp
    Mult = mybir.AluOpType.mult
    Add = mybir.AluOpType.add

    nc = tc.nc

    with ExitStack() as ctx:
        consts = ctx.enter_context(tc.tile_pool(name="consts", bufs=1))
        vtpool = ctx.enter_context(
            tc.tile_pool(name="vtpool", bufs=vpool_bufs))
        thpool = ctx.enter_context(tc.tile_pool(name="thpool", bufs=18))
        wpool = ctx.enter_context(tc.tile_pool(name="wpool", bufs=2))
        bpool = ctx.enter_context(tc.tile_pool(name="bpool", bufs=2))
        scrpool = ctx.enter_context(tc.tile_pool(name="scrpool", bufs=2))
        opool = ctx.enter_context(tc.tile_pool(name="opool", bufs=4))
        ps_vp = ctx.enter_context(
            tc.tile_pool(name="ps_vp", bufs=4, space="PSUM"))
        ps_sm = ctx.enter_context(
            tc.tile_pool(name="ps_sm", bufs=4, space="PSUM"))

        # GPSIMD ucode library providing partition_broadcast
        from concourse import library_config
        nc.gpsimd.load_library(library_config.mlp)

        # --- constants ---
        w2_sb = consts.tile([P, EC, D], dtm)
        nc.sync.dma_start(w2_sb, w2ed_d.rearrange("(c p) d -> p c d", p=P))
        cb_sb = consts.tile([P, DC, b_loc], f32)
        nc.sync.dma_start(cb_sb, cb_d.rearrange("(c p) b -> p c b", p=P))
        v_sb = consts.tile([P, DC, 1], dtm)
        nc.sync.dma_start(v_sb, v_d.rearrange("(c p) x -> p c x", p=P))

        def emit_weights(p):
            """Prev group's exp + den partials (ACT) -> broadcast rows.

            partition_broadcast reads physical partition 0 of its source
            (AP partition offsets are ignored), so every score row lives at
            partition 0 of its own psum bank / SBUF tile."""
            psss, b, g, rep = p
            denp = opool.tile([1, SUP_PER_GROUP], f32, tag="dp",
                              name=f"dp_{rep}_{b}_{g}")
            wb = bpool.tile([P, SUP_PER_GROUP, TSUP], dtm, tag="wb",
                            name=f"wb_{rep}_{b}_{g}")

            def half(srange):
                for s in srange:
                    wrow = wpool.tile([1, TSUP], dtm, tag=f"wr{s}",
                                      name=f"wr{s}_{rep}_{b}_{g}")
                    nc.scalar.activation(wrow, psss[s], Exp,
                                         accum_out=denp[:, s:s + 1])
                    nc.gpsimd.partition_broadcast(
                        wb[:, s, :], wrow, channels=P)
            return wb, denp, half

        def emit_numerator(p, wb, denp):
            """Prev group's numerator: 4 DVE fused mult+reduce over t."""
            vt, b, g, rep = p["vt"], p["b"], p["g"], p["rep"]
            osb = opool.tile([P, OUTW], f32, tag="osb",
                             name=f"osb_{rep}_{b}_{g}")
            scr = scrpool.tile([P, T_GROUP], dtm, tag="scr",
                               name=f"scr_{rep}_{b}_{g}")
            for c in range(EC):
                nc.vector.scalar_tensor_tensor(
                    out=scr, in0=vt[:, c, :], scalar=1.0,
                    in1=wb.rearrange("p s t -> p (s t)"),
                    op0=Mult, op1=Mult,
                    accum_out=osb[:, c:c + 1])
            nc.vector.tensor_copy(osb[0:1, EC:], denp)
            # Output DMA on the Activation hwdge queue: on the SP queue its
            # semaphore wait (DVE numerator) head-of-line-blocks the next
            # groups' vt chunk loads.
            nc.scalar.dma_start(outp[b, g], osb)

        def emit_scores(p):
            """Prev group's score MMs: deferred a full group so the PE
            never waits on that group's tanh latency."""
            for s in range(SUP_PER_GROUP):
                nc.tensor.matmul(p["psss"][s], lhsT=v_sb[:, 0, :],
                                 rhs=p["ths"][s][0], start=True, stop=False)
                nc.tensor.matmul(p["psss"][s], lhsT=v_sb[:, 1, :],
                                 rhs=p["ths"][s][1], start=False, stop=True)

        def body(rep):
          prev = None
          for b in range(b_loc):
            for g in range(n_groups):
                t0g = g * T_GROUP
                vt = vtpool.tile([P, EC, T_GROUP], dtm, tag="vt",
                                 name=f"vt_{rep}_{b}_{g}")
                for c in range(EC):
                    nc.sync.dma_start(
                        vt[:, c, :],
                        valsT[b, c * P:(c + 1) * P,
                              t0g:t0g + T_GROUP])

                # scores psum: one single-row bank per super (row 0, so
                # the downstream partition_broadcast sees partition 0)
                psss = [ps_sm.tile([1, TSUP], f32, tag="scrow",
                                   name=f"pss{s}_{rep}_{b}_{g}")
                        for s in range(SUP_PER_GROUP)]

                # ---- v_proj, dc-outer so each W2 chunk is loaded into the
                # PE once per pass and reused for 4 super-matmuls ----------
                ths = [[None] * DC for _ in range(SUP_PER_GROUP)]
                psvs = [None] * SUP_PER_GROUP
                # Prev group's deferred score MMs first: they fill the
                # PE while this group's first psv bank waits on the prev
                # dc1 tanh (their own tanh inputs are a full group old).
                if prev is not None:
                    emit_scores(prev)
                for dc in range(DC):
                    for c in range(EC):
                        for s in range(SUP_PER_GROUP):
                            if c == 0:
                                psvs[s] = ps_vp.tile(
                                    [P, TSUP], f32, tag="psv",
                                    name=f"psv_{rep}_{b}_{g}_{s}_{dc}")
                            nc.tensor.matmul(
                                psvs[s],
                                lhsT=w2_sb[:, c, dc * P:(dc + 1) * P],
                                rhs=vt[:, c, s * TSUP:(s + 1) * TSUP],
                                start=(c == 0), stop=(c == EC - 1))
                            if c == EC - 1:
                                th = thpool.tile(
                                    [P, TSUP], dtm, tag="th",
                                    name=f"th_{rep}_{b}_{g}_{s}_{dc}")
                                nc.scalar.activation(
                                    th, psvs[s], Tanh,
                                    bias=cb_sb[:, dc, b:b + 1])
                                ths[s][dc] = th
                    # Prev group's softmax tail: half the exps sit between
                    # the two weight passes (so the numerator chain starts),
                    # half after dc1's tanhs (so those tanhs are not queued
                    # behind 4 exps - the next group's first psv bank waits
                    # on them).
                    if prev is not None:
                        if dc == 0:
                            wb_p, denp_p, half_p = emit_weights(prev["soft"])
                            half_p(range(0, 2))
                        else:
                            half_p(range(2, SUP_PER_GROUP))
                            emit_numerator(prev, wb_p, denp_p)
                            prev = None

                prev = {"vt": vt, "b": b, "g": g, "rep": rep, "ths": ths,
                        "psss": psss, "soft": (psss, b, g, rep)}
          emit_scores(prev)
          wb_p, denp_p, half_p = emit_weights(prev["soft"])
          half_p(range(SUP_PER_GROUP))
          emit_numerator(prev, wb_p, denp_p)

        if loop_n > 1:
            with tc.For_i(0, loop_n, 1):
                for rep in range(repeat):
                    body(rep)
        else:
            for rep in range(repeat):
                body(rep)


def host_prepare(values, query, v, W1_w, W1_b, W2_w, W2_b, b_loc=B_LOC,
                 n_cores=N_CORES):
    """Precompute tiny host-side tensors and build per-core input maps."""
    import ml_dtypes

    npm = ml_dtypes.bfloat16

    c = (query.astype(np.float32) @ W1_w.T.astype(np.float32)
         + W1_b + W2_b).astype(np.float32)          # [B, D]
    values_m = np.asarray(values).astype(npm)
    values_t = np.ascontiguousarray(values_m.transpose(0, 2, 1))  # [B, E, T]
    w2ed = np.ascontiguousarray(np.asarray(W2_w).T.astype(npm))  # [E, D]
    vcol = np.ascontiguousarray(np.asarray(v).reshape(D, 1).astype(npm))
    in_maps = []
    for k in range(n_cores):
        bsl = slice(k * b_loc, (k + 1) * b_loc)
        in_maps.append({
            "valuesT": np.ascontiguousarray(values_t[bsl]),
            "w2ed": w2ed,
            "cb": np.ascontiguousarray(c[bsl].T),    # [D, b_loc]
            "vcol": vcol,
        })
    return in_maps


def host_combine(results, b_loc=B_LOC, n_cores=N_CORES):
    """Combine per-(batch, group) partial numerators/denominators.

    out_parts[b, g, p, c] for c<EC is sum_t w[t] * values[t, c*128+p];
    row 0 cols EC..EC+3 hold the per-super partial denominators.
    No max-subtraction: partials are exact exp sums (safe in f32 range).
    """
    out = np.zeros((n_cores * b_loc, E), np.float32)
    for k in range(n_cores):
        parts = np.asarray(results[k]["out_parts"]).astype(np.float64)
        num = parts[..., :EC]                       # [b, g, 128, EC]
        den = parts[..., 0, EC:].sum(-1)            # [b, g]
        numf = num.transpose(0, 1, 3, 2).reshape(b_loc, -1, E)  # e=c*128+p
        o = numf.sum(1) / den.sum(1)[:, None]
        out[k * b_loc:(k + 1) * b_loc] = o.astype(np.float32)
    return out


_NC_CACHE = None


def kernel(values, query, v, W1_w, W1_b, W2_w, W2_b):
    global _NC_CACHE, LAST_RESULT
    from concourse.bass_utils import run_bass_kernel_spmd

    in_maps = host_prepare(values, query, v, W1_w, W1_b, W2_w, W2_b)
    if _NC_CACHE is None:
        _NC_CACHE = build_bass()
    trace = bool(int(os.environ.get("KERNEL_TRACE", "0")))
    LAST_RESULT = run_bass_kernel_spmd(
        _NC_CACHE, in_maps, list(range(N_CORES)), trace=trace)
    return host_combine(LAST_RESULT.results)


if __name__ == "__main__":
    rng = np.random.default_rng(0)
    inputs = {
        "values": rng.standard_normal((B, T, E), dtype=np.float32),
        "query": rng.standard_normal((B, D), dtype=np.float32),
        "v": rng.random(D, dtype=np.float32),
        "W1_w": rng.standard_normal((D, D), dtype=np.float32) * 0.06,
        "W1_b": rng.standard_normal(D, dtype=np.float32) * 0.06,
        "W2_w": rng.standard_normal((D, E), dtype=np.float32) * 0.04,
        "W2_b": rng.standard_normal(D, dtype=np.float32) * 0.04,
    }
    t0 = time.time()
    out = kernel(**inputs)
    print("kernel done in", time.time() - t0, "s", out.shape, out.dtype)
